# revision 3
# baseline (speedup 1.0000x reference)
"""Trainium2 Bass kernel for nn_CircuitRankNet (2-layer GCN siamese + mean-pool + MLP).

Algebraic collapse: the two GCN layers have no nonlinearity between them, so
with M = D^-1/2 (A+I) D^-1/2 the pooled embeddings only need
    P = (M^T M^T B)^T X   (B = one-hot(batch) [N, 64])
Folding the norms:  Chat[i,:] = dinv_i^2 * sum_{e: src=i} dinv_dst * onehot64(batch[dst])
                    Xhat[j,:] = dinv_j * X[j,:]
    P[g, d] = sum_over_aug_edges  Chat[dst_e, g] * Xhat[src_e, d]

Sharding: nodes (and their incident edges, by dst) are split into 8 contiguous
ranges, one per NeuronCore. Each core owns the Chat rows for its range, so
pass-2 gathers are core-local; the per-core partial P [2,64,128] is summed on
the host and fed to the tiny compare MLP.

Device kernel (per core, per graph side): iterate 64-node windows of the local
range; gather the window's Chat rows once; for each 128-edge sub-block gather
Xhat[src] rows, build the dst-slot one-hot via an iota compare, and
segment-sum with a matmul (XS += oh^T @ xexp); finally accumulate
P += Chat_w^T-style matmul (lhsT=Chat_w, rhs=XS) in PSUM across all windows.
"""
import numpy as np

NCORES = 8
N = 100000
E = 1600000
G = 64
DIN = 128
DH = 128

WSPAN = 56        # nodes per window (<= 64 slots)
SUBS = 8          # sub-blocks (of 128 edges) per window
WCAP = SUBS * 128 # edge capacity per window
PAD_LR = 65.0     # slot value that never matches iota 0..63

_cache = {}


def _preprocess_side(x, edge_index, batch):
    src = np.asarray(edge_index[0], np.int64)
    dst = np.asarray(edge_index[1], np.int64)
    batch = np.asarray(batch, np.int64)
    x = np.asarray(x, np.float32)

    deg = np.bincount(dst, minlength=N).astype(np.float64) + 1.0
    dinv = (1.0 / np.sqrt(deg)).astype(np.float32)

    sl = np.arange(N, dtype=np.int64)
    asrc = np.concatenate([src, sl])
    adst = np.concatenate([dst, sl])

    norm64 = (dinv[asrc].astype(np.float64) * dinv[adst].astype(np.float64))
    t_g = np.bincount(batch[adst], weights=norm64, minlength=G)
    n_g = np.bincount(batch, minlength=G).astype(np.float64)

    # Chat rows (host): Chat[i,g] = dinv_i^2 * sum_{e:src=i} dinv_dst * [batch_dst==g]
    w = dinv[asrc] * dinv[asrc] * dinv[adst]
    chat = np.bincount(asrc * G + batch[adst], weights=w.astype(np.float64),
                       minlength=N * G).reshape(N, G).astype(np.float32)

    xhat = dinv[:, None] * x

    # core ranges balanced by incident-edge (dst) counts
    indeg = np.bincount(adst, minlength=N)
    cum = np.cumsum(indeg)
    targets = np.arange(1, NCORES) * (cum[-1] / NCORES)
    bounds = np.searchsorted(cum, targets)
    node_lo = np.concatenate([[0], bounds + 1]).astype(np.int64)
    node_hi = np.concatenate([bounds + 1, [N]]).astype(np.int64)

    order = np.argsort(adst, kind="stable")
    asrc_s, adst_s = asrc[order], adst[order]
    # edge range per core in the dst-sorted list
    core_e0 = np.searchsorted(adst_s, node_lo)
    core_e1 = np.searchsorted(adst_s, node_hi)

    cores = []
    for c in range(NCORES):
        lo, hi = int(node_lo[c]), int(node_hi[c])
        es, ee = int(core_e0[c]), int(core_e1[c])
        s2, d2 = asrc_s[es:ee], adst_s[es:ee]
        # windows: fixed WSPAN-node spans, split when over edge capacity
        wlo_list, we0, we1 = [], [], []
        wstart = lo
        while wstart < hi:
            wend = min(wstart + WSPAN, hi)
            a = es + np.searchsorted(d2, wstart)
            b = es + np.searchsorted(d2, wend)
            for ws in range(a, b, WCAP):
                wlo_list.append(wstart)
                we0.append(ws)
                we1.append(min(ws + WCAP, b))
            if a == b:  # empty window still emitted (keeps layout simple)
                wlo_list.append(wstart)
                we0.append(a)
                we1.append(a)
            wstart = wend
        cores.append(dict(lo=lo, hi=hi, wlo=np.asarray(wlo_list),
                          we0=np.asarray(we0) - es, we1=np.asarray(we1) - es,
                          src=s2, dst=d2))
    nloc_max = int((node_hi - node_lo).max())
    return dict(cores=cores, chat=chat, xhat=xhat, t_g=t_g, n_g=n_g,
                nloc_max=nloc_max)


def _pack_core(core, chat, nw_max, nloc_max):
    """-> xidx [nw,128,8] i32, lr [nw,128,8] f32, cidx [nw,128] i32, chat_loc."""
    lo, hi = core["lo"], core["hi"]
    nw = len(core["wlo"])
    xidx = np.zeros((nw_max, 128, SUBS), np.int32)
    lr = np.full((nw_max, 128, SUBS), PAD_LR, np.float32)
    cidx = np.zeros((nw_max, 128), np.int32)
    for wi in range(nw):
        e0, e1 = core["we0"][wi], core["we1"][wi]
        ne = e1 - e0
        if ne:
            jj = np.arange(ne)
            b, p = jj // 128, jj % 128
            xidx[wi, p, b] = core["src"][e0:e1]
            lr[wi, p, b] = (core["dst"][e0:e1] - core["wlo"][wi]).astype(np.float32)
        cidx[wi, :] = np.minimum(core["wlo"][wi] - lo + np.arange(128), nloc_max - 1)
    chat_loc = np.zeros((nloc_max, G), np.float32)
    chat_loc[: hi - lo] = chat[lo:hi]
    return xidx, lr, cidx, chat_loc


def _build_nc(nw_max, nloc_max):
    import concourse.bass as bass
    import concourse.bacc as bacc
    import concourse.mybir as mybir
    import concourse.tile as tile

    nc = bacc.Bacc("TRN2", target_bir_lowering=False, debug=False,
                   num_devices=NCORES)
    f32, i32 = mybir.dt.float32, mybir.dt.int32

    xh = [nc.dram_tensor(f"xh{s}", [N, DIN], f32, kind="ExternalInput")
          for s in range(2)]
    ch = [nc.dram_tensor(f"chat{s}", [nloc_max, G], f32, kind="ExternalInput")
          for s in range(2)]
    xidx = [nc.dram_tensor(f"xidx{s}", [nw_max, 128, SUBS], i32, kind="ExternalInput")
            for s in range(2)]
    lrt = [nc.dram_tensor(f"lr{s}", [nw_max, 128, SUBS], f32, kind="ExternalInput")
           for s in range(2)]
    cidx = [nc.dram_tensor(f"cidx{s}", [nw_max, 128], i32, kind="ExternalInput")
            for s in range(2)]
    iota = nc.dram_tensor("iota", [128, SUBS * G], f32, kind="ExternalInput")
    pout = [nc.dram_tensor(f"P{s}", [G, DIN], f32, kind="ExternalOutput")
            for s in range(2)]

    with tile.TileContext(nc) as tc:
        with tc.tile_pool(name="const", bufs=1) as cpool, \
             tc.tile_pool(name="meta", bufs=3) as mpool, \
             tc.tile_pool(name="xe", bufs=6) as xpool, \
             tc.tile_pool(name="work", bufs=3) as wpool, \
             tc.tile_pool(name="acc", bufs=1) as apool, \
             tc.tile_pool(name="xsp", bufs=2, space="PSUM") as xspool, \
             tc.tile_pool(name="pp", bufs=2, space="PSUM") as ppool:
            it = cpool.tile([128, SUBS * G], f32)
            nc.sync.dma_start(out=it[:], in_=iota[:, :])
            for s in range(2):
                pacc = ppool.tile([G, DIN], f32)
                for w in range(nw_max):
                    xi = mpool.tile([128, SUBS], i32, tag="xi")
                    nc.sync.dma_start(out=xi[:], in_=xidx[s][w, :, :])
                    lw = mpool.tile([128, SUBS], f32, tag="lw")
                    nc.sync.dma_start(out=lw[:], in_=lrt[s][w, :, :])
                    ci = mpool.tile([128, 1], i32, tag="ci")
                    nc.sync.dma_start(out=ci[:], in_=cidx[s][w, :, None])
                    cw = wpool.tile([128, G], f32, tag="cw")
                    nc.gpsimd.indirect_dma_start(
                        out=cw[:], out_offset=None, in_=ch[s][:, :],
                        in_offset=bass.IndirectOffsetOnAxis(ap=ci[:, :], axis=0))
                    # one-hot for all 8 sub-blocks in one op
                    oh = wpool.tile([128, SUBS * G], f32, tag="oh")
                    lwb = lw[:].rearrange("p (b o) -> p b o", o=1) \
                               .to_broadcast([128, SUBS, G])
                    nc.vector.tensor_tensor(
                        out=oh[:].rearrange("p (b g) -> p b g", g=G),
                        in0=it[:].rearrange("p (b g) -> p b g", g=G),
                        in1=lwb, op=mybir.AluOpType.is_equal)
                    xs = xspool.tile([G, DIN], f32)
                    for b in range(SUBS):
                        xe = xpool.tile([128, DIN], f32, tag="xe")
                        nc.gpsimd.indirect_dma_start(
                            out=xe[:], out_offset=None, in_=xh[s][:, :],
                            in_offset=bass.IndirectOffsetOnAxis(
                                ap=xi[:, b:b + 1], axis=0))
                        nc.tensor.matmul(
                            out=xs[:, :], lhsT=oh[:, b * G:(b + 1) * G],
                            rhs=xe[:, :], start=(b == 0), stop=(b == SUBS - 1))
                    xsb = wpool.tile([G, DIN], f32, tag="xsb")
                    nc.vector.tensor_copy(out=xsb[:], in_=xs[:, :])
                    nc.tensor.matmul(
                        out=pacc[:, :], lhsT=cw[:G, :], rhs=xsb[:, :],
                        start=(w == 0), stop=(w == nw_max - 1))
                pf = apool.tile([G, DIN], f32, tag="pf")
                nc.vector.tensor_copy(out=pf[:], in_=pacc[:, :])
                nc.sync.dma_start(out=pout[s][:, :], in_=pf[:])
    nc.compile()
    return nc


def last_nc_and_inmaps():
    return kernel.last_nc, kernel.last_in_maps


def kernel(x0, x1, edge_index0, edge_index1, batch0, batch1,
           W1, b1, W2, b2, Wc1, bc1, Wc2, bc2):
    from concourse import bass_utils

    prep0 = _preprocess_side(x0, edge_index0, batch0)
    prep1 = _preprocess_side(x1, edge_index1, batch1)

    nw_max = max(max(len(c["wlo"]) for c in prep0["cores"]),
                 max(len(c["wlo"]) for c in prep1["cores"]))
    nloc_max = max(prep0["nloc_max"], prep1["nloc_max"])

    key = (nw_max, nloc_max)
    if key not in _cache:
        _cache[key] = _build_nc(nw_max, nloc_max)
    nc = _cache[key]

    iota = np.tile(np.arange(G, dtype=np.float32)[None, :], (128, SUBS))
    in_maps = []
    for c in range(NCORES):
        m = dict(iota=iota)
        for s, prep in ((0, prep0), (1, prep1)):
            xidx, lr, cidx, chat_loc = _pack_core(
                prep["cores"][c], prep["chat"], nw_max, nloc_max)
            m[f"xh{s}"] = np.ascontiguousarray(prep["xhat"])
            m[f"chat{s}"] = chat_loc
            m[f"xidx{s}"] = xidx
            m[f"lr{s}"] = lr
            m[f"cidx{s}"] = cidx
        in_maps.append(m)

    kernel.last_nc = nc
    kernel.last_in_maps = in_maps
    res = bass_utils.run_bass_kernel_spmd(nc, in_maps, core_ids=list(range(NCORES)))
    kernel.last_results = res

    P0 = np.zeros((G, DIN), np.float64)
    P1 = np.zeros((G, DIN), np.float64)
    for c in range(NCORES):
        P0 += res.results[c]["P0"]
        P1 += res.results[c]["P1"]

    # host finish: tiny pooled + compare MLP (4 MFLOP)
    W1 = np.asarray(W1, np.float32); W2 = np.asarray(W2, np.float32)
    Wp = W1 @ W2
    bp1 = np.asarray(b1, np.float32) @ W2

    def pooled(P, t, n):
        out = (P.astype(np.float32) @ Wp + t[:, None].astype(np.float32) * bp1[None, :]
               + n[:, None].astype(np.float32) * np.asarray(b2, np.float32)[None, :])
        return out / np.maximum(n, 1.0)[:, None].astype(np.float32)

    cfeat = np.concatenate([pooled(P0, prep0["t_g"], prep0["n_g"]),
                            pooled(P1, prep1["t_g"], prep1["n_g"])], axis=1)
    h = 1.0 / (1.0 + np.exp(-(cfeat @ np.asarray(Wc1, np.float32)
                              + np.asarray(bc1, np.float32))))
    prob = 1.0 / (1.0 + np.exp(-(h @ np.asarray(Wc2, np.float32)
                                 + np.asarray(bc2, np.float32))))
    return prob[:, 0].astype(np.float32)



# revision 6
# speedup vs baseline: 25.7981x; 25.7981x over previous
"""Trainium2 Bass kernel for nn_CircuitRankNet (2-layer GCN siamese + mean-pool + MLP).

Algebraic collapse: the two GCN layers have no nonlinearity between them, so
with M = D^-1/2 (A+I) D^-1/2 the pooled embeddings only need
    P = B^T M M X   (B = one-hot(batch) [N, 64])
Folding the norms:  Chat[i,:] = dinv_i^2 * sum_{e: src=i} dinv_dst * onehot64(batch[dst])
                    Xhat[j,:] = dinv_j * X[j,:]
    P[g, d] = sum_over_aug_edges  Chat[dst_e, g] * Xhat[src_e, d]

Device kernel (per core, per side): edges are sharded by dst range (12500
nodes per core) and sorted by (src_chunk, dst). In superblocks of 8192 edges,
two batched dma_gather calls (round-robin over 4 SWDGE queues — the gather is
per-request bound at ~3ns/row, queues parallelize descriptor processing)
fetch CG = Chat[dst] rows (per-core-local, int16 index safe) and XE =
Xhat[src] rows (per 25000-row src chunk, int16 safe); then one matmul per
128-edge column accumulates PT[d, g] += XE_col^T @ CG_col in PSUM across the
whole side. Everything bf16 (tolerance 2e-2); no indirect DMA (per-row Q7
descriptor generation made the old kernel ~98ms), no data-dependent module
structure (pads gather a zero Chat row).

Host does the cheap first hop (Chat via bincount), the edge sort/packing, the
final tiny pooled+compare MLP, and the 8-way partial-P reduction.
"""
import numpy as np
import ml_dtypes

NCORES = 8
N = 100000
E = 1600000
G = 64
DIN = 128
DH = 128

NLOC = N // NCORES          # dst nodes per core (12500)
NCHUNK = 4
CHUNK = N // NCHUNK         # xhat rows per chunk (25000 < 32768 int16 limit)
SB = 8192                   # edges per dma_gather call
COLS = SB // 128            # matmul columns per superblock (64)
NQ = 4                      # SWDGE queues

BF16 = ml_dtypes.bfloat16

_cache = {}


def _preprocess_side(x, edge_index, batch):
    src = np.asarray(edge_index[0], np.int64)
    dst = np.asarray(edge_index[1], np.int64)
    batch = np.asarray(batch, np.int64)
    x = np.asarray(x, np.float32)

    deg = np.bincount(dst, minlength=N).astype(np.float64) + 1.0
    dinv = (1.0 / np.sqrt(deg)).astype(np.float32)

    sl = np.arange(N, dtype=np.int64)
    asrc = np.concatenate([src, sl])
    adst = np.concatenate([dst, sl])

    norm64 = dinv[asrc].astype(np.float64) * dinv[adst].astype(np.float64)
    t_g = np.bincount(batch[adst], weights=norm64, minlength=G)
    n_g = np.bincount(batch, minlength=G).astype(np.float64)

    # first hop on host: Chat[i,g] = dinv_i^2 * sum_{e:src=i} dinv_dst [batch_dst=g]
    w = dinv[asrc] * dinv[asrc] * dinv[adst]
    chat = np.bincount(asrc * G + batch[adst], weights=w.astype(np.float64),
                       minlength=N * G).reshape(N, G).astype(np.float32)

    xhat = (dinv[:, None] * x).astype(BF16)

    # edge order: (dst core, src chunk, dst) -- dst-sorted within segment
    core = adst // NLOC
    chunk = asrc // CHUNK
    dloc = adst - core * NLOC
    key = ((core * NCHUNK + chunk) << 14) | dloc
    order = np.argsort(key, kind="stable")
    seg_id = (core * NCHUNK + chunk)[order]
    seg_counts = np.bincount(seg_id, minlength=NCORES * NCHUNK)
    sloc = (asrc - chunk * CHUNK)[order].astype(np.int16)
    dloc = dloc[order].astype(np.int16)
    return dict(chat=chat, xhat=xhat, t_g=t_g, n_g=n_g,
                sloc=sloc, dloc=dloc, seg_counts=seg_counts)


def _pack_idx(vals, pad_val, nsb):
    """int16 edge values -> dma_gather index tiles [nsb, 128, SB//16]."""
    out = np.full(nsb * SB, pad_val, np.int16)
    out[: len(vals)] = vals
    t = out.reshape(nsb, SB // 16, 16).transpose(0, 2, 1)  # [nsb, 16, SB//16]
    return np.tile(t, (1, 8, 1))


def _pack_core(prep, c, nsb_seg):
    """-> xi [4*nsb_seg, 128, SB//16], ci same, chat_loc [NLOC+1, 128] bf16."""
    nsb_tot = NCHUNK * nsb_seg
    xi = np.empty((nsb_tot, 128, SB // 16), np.int16)
    ci = np.empty((nsb_tot, 128, SB // 16), np.int16)
    offs = np.concatenate([[0], np.cumsum(prep["seg_counts"])])
    for k in range(NCHUNK):
        seg = c * NCHUNK + k
        a, b = offs[seg], offs[seg + 1]
        r = slice(k * nsb_seg, (k + 1) * nsb_seg)
        xi[r] = _pack_idx(prep["sloc"][a:b], 0, nsb_seg)
        ci[r] = _pack_idx(prep["dloc"][a:b], NLOC, nsb_seg)
    chat_loc = np.zeros((NLOC + 1, 128), BF16)
    chat_loc[:NLOC, :G] = prep["chat"][c * NLOC:(c + 1) * NLOC]
    return xi, ci, chat_loc


def _build_nc(nsb_seg, reps=1):
    import concourse.bass as bass
    import concourse.bacc as bacc
    import concourse.mybir as mybir
    import concourse.tile as tile
    from contextlib import nullcontext

    nc = bacc.Bacc("TRN2", target_bir_lowering=False, debug=False,
                   num_devices=NCORES, num_swdge_queues=NQ)
    f32, i16, bf16 = mybir.dt.float32, mybir.dt.int16, mybir.dt.bfloat16
    nsb_tot = NCHUNK * nsb_seg

    xh = [nc.dram_tensor(f"xh{s}", [N, DIN], bf16, kind="ExternalInput")
          for s in range(2)]
    ch = [nc.dram_tensor(f"ch{s}", [NLOC + 1, 128], bf16, kind="ExternalInput")
          for s in range(2)]
    xi = [nc.dram_tensor(f"xi{s}", [nsb_tot, 128, SB // 16], i16,
                         kind="ExternalInput") for s in range(2)]
    ci = [nc.dram_tensor(f"ci{s}", [nsb_tot, 128, SB // 16], i16,
                         kind="ExternalInput") for s in range(2)]
    pout = [nc.dram_tensor(f"PT{s}", [DIN, G], f32, kind="ExternalOutput")
            for s in range(2)]

    with tile.TileContext(nc) as tc:
        with tc.tile_pool(name="idx", bufs=3) as ipool, \
             tc.tile_pool(name="gat", bufs=3) as gpool, \
             tc.tile_pool(name="out", bufs=1) as opool, \
             tc.tile_pool(name="pp", bufs=2, space="PSUM") as ppool:
          with (tc.For_i(0, reps) if reps > 1 else nullcontext()):
            q = 0
            for s in range(2):
                pt = ppool.tile([DIN, G], f32)
                nmm = nsb_tot * COLS
                mm = 0
                for k in range(NCHUNK):
                    xh_chunk = xh[s][k * CHUNK:(k + 1) * CHUNK, :]
                    for j in range(nsb_seg):
                        sb = k * nsb_seg + j
                        xit = ipool.tile([128, SB // 16], i16, tag="xi")
                        nc.sync.dma_start(out=xit[:], in_=xi[s][sb, :, :])
                        cit = ipool.tile([128, SB // 16], i16, tag="ci")
                        nc.sync.dma_start(out=cit[:], in_=ci[s][sb, :, :])
                        xe = gpool.tile([128, COLS, DIN], bf16, tag="xe")
                        nc.gpsimd.dma_gather(
                            xe[:], xh_chunk, xit[:], SB, SB, DIN,
                            single_packet=False, queue_num=q % NQ)
                        q += 1
                        cg = gpool.tile([128, COLS, 128], bf16, tag="cg")
                        nc.gpsimd.dma_gather(
                            cg[:], ch[s][:, :], cit[:], SB, SB, 128,
                            single_packet=False, queue_num=q % NQ)
                        q += 1
                        for col in range(COLS):
                            lhsT = xe[:, col:col + 1, :].rearrange(
                                "p a b -> p (a b)")
                            rhs = cg[:, col:col + 1, :G].rearrange(
                                "p a b -> p (a b)")
                            nc.tensor.matmul(
                                out=pt[:, :], lhsT=lhsT, rhs=rhs,
                                start=(mm == 0), stop=(mm == nmm - 1))
                            mm += 1
                pf = opool.tile([DIN, G], f32, tag=f"pf{s}")
                nc.vector.tensor_copy(out=pf[:], in_=pt[:, :])
                nc.sync.dma_start(out=pout[s][:, :], in_=pf[:])
    nc.compile()
    return nc


def build_bench_nc(reps):
    """Same kernel wrapped in a For_i(0, reps) hardware loop, for RTT-free
    device-time measurement: T = (wall(reps) - wall(1)) / (reps - 1)."""
    nsb_seg = kernel.last_nsb_seg
    key = (nsb_seg, reps)
    if key not in _cache:
        _cache[key] = _build_nc(nsb_seg, reps)
    return _cache[key]


def last_nc_and_inmaps():
    return kernel.last_nc, kernel.last_in_maps


def kernel(x0, x1, edge_index0, edge_index1, batch0, batch1,
           W1, b1, W2, b2, Wc1, bc1, Wc2, bc2):
    from concourse import bass_utils

    prep0 = _preprocess_side(x0, edge_index0, batch0)
    prep1 = _preprocess_side(x1, edge_index1, batch1)

    nsb_seg = 0
    for prep in (prep0, prep1):
        nsb_seg = max(nsb_seg, -(-int(prep["seg_counts"].max()) // SB))

    key = (nsb_seg, 1)
    if key not in _cache:
        _cache[key] = _build_nc(nsb_seg)
    nc = _cache[key]

    in_maps = []
    for c in range(NCORES):
        m = {}
        for s, prep in ((0, prep0), (1, prep1)):
            xi, ci, chat_loc = _pack_core(prep, c, nsb_seg)
            m[f"xh{s}"] = prep["xhat"]
            m[f"ch{s}"] = chat_loc
            m[f"xi{s}"] = xi
            m[f"ci{s}"] = ci
        in_maps.append(m)

    kernel.last_nc = nc
    kernel.last_in_maps = in_maps
    kernel.last_nsb_seg = nsb_seg
    res = bass_utils.run_bass_kernel_spmd(nc, in_maps, core_ids=list(range(NCORES)))
    kernel.last_results = res

    P0 = np.zeros((G, DIN), np.float64)
    P1 = np.zeros((G, DIN), np.float64)
    for c in range(NCORES):
        P0 += res.results[c]["PT0"].astype(np.float64).T
        P1 += res.results[c]["PT1"].astype(np.float64).T

    # host finish: tiny pooled + compare MLP (4 MFLOP)
    W1 = np.asarray(W1, np.float32); W2 = np.asarray(W2, np.float32)
    Wp = W1 @ W2
    bp1 = np.asarray(b1, np.float32) @ W2

    def pooled(P, t, n):
        out = (P.astype(np.float32) @ Wp + t[:, None].astype(np.float32) * bp1[None, :]
               + n[:, None].astype(np.float32) * np.asarray(b2, np.float32)[None, :])
        return out / np.maximum(n, 1.0)[:, None].astype(np.float32)

    cfeat = np.concatenate([pooled(P0, prep0["t_g"], prep0["n_g"]),
                            pooled(P1, prep1["t_g"], prep1["n_g"])], axis=1)
    h = 1.0 / (1.0 + np.exp(-(cfeat @ np.asarray(Wc1, np.float32)
                              + np.asarray(bc1, np.float32))))
    prob = 1.0 / (1.0 + np.exp(-(h @ np.asarray(Wc2, np.float32)
                                 + np.asarray(bc2, np.float32))))
    return prob[:, 0].astype(np.float32)
